# revision 16
# baseline (speedup 1.0000x reference)
# Binary (sign) matmul: out[b,m,n] = sum_k sign(x[b,m,k]) * sign(y[b,n,k]) * x_clip * y_clip
# B=2, M=N=K=4096, fp32 in/out.
#
# Sharding: 8 cores = batch(2) x 2x2 grid over (M, N). Each core computes a
# [2048, 2048] output block from x[b, mh*2048:, :] and y[b, nh*2048:, :].
# The host binds each core's shards in k-major (transposed) layout — pure
# input marshalling; all arithmetic (sign, matmul, clip scaling) runs on
# device.
#
# Per-core device pipeline (all engines overlapped, no phase barriers):
#   DMA fp32 k-major tiles -> ScalarE Sign (fp32 -> fp8e4 +-1, written
#   straight into the matmul operand buffers) -> TensorE DoubleRow fp8
#   matmuls (exact: sums of +-1 accumulate in fp32 PSUM) -> DVE scale by
#   x_clip*y_clip (computed on device) -> DMA out.
import numpy as np

B = 2
M = N = K = 4096
P = 128
MSH, NSH = 2048, 2048      # per-core shard of M, N
KO = K // P                # 32 k-tiles of 128
MT = MSH // P              # 16 m row-tiles
FD = 512                   # matmul free dim
NCH = NSH // FD            # 4 n chunks
NCORES = 8

USE_FP8 = True             # fp8e4 operands + DoubleRow perf mode


def _build_program():
    import concourse.bacc as bacc
    import concourse.mybir as mybir
    import concourse.tile as tile
    from concourse.bass import ts

    f32 = mybir.dt.float32
    bf16 = mybir.dt.bfloat16
    op_dt = mybir.dt.float8e4 if USE_FP8 else bf16
    Sign = mybir.ActivationFunctionType.Sign

    # Bacc (not bass.Bass): its compile() legalizes multi-sem waits into
    # event-semaphore carriers — TRN2 instructions support only 1 HW wait.
    nc = bacc.Bacc(
        "TRN2",
        target_bir_lowering=False,
        debug=False,
        num_devices=NCORES,
    )
    xsT = nc.dram_tensor("xsT", [K, MSH], f32, kind="ExternalInput").ap()
    ysT = nc.dram_tensor("ysT", [K, NSH], f32, kind="ExternalInput").ap()
    clips = nc.dram_tensor("clips", [P, 2], f32, kind="ExternalInput").ap()
    out = nc.dram_tensor("out", [MSH, NSH], f32, kind="ExternalOutput").ap()

    with tile.TileContext(nc) as tc:
        with (
            tc.tile_pool(name="constp", bufs=1) as constp,
            tc.tile_pool(name="sytp", bufs=1) as sytp,
            tc.tile_pool(name="sxtp", bufs=1) as sxtp,
            tc.tile_pool(name="stagep", bufs=6) as stagep,
            tc.tile_pool(name="outp", bufs=4) as outp,
            tc.tile_pool(name="psump", bufs=8, space="PSUM") as psump,
        ):
            # clip product, replicated per-partition: [P, 1]
            clip_sb = constp.tile([P, 2], f32)
            nc.sync.dma_start(clip_sb[:], clips)
            clip_prod = constp.tile([P, 1], f32)
            nc.vector.tensor_tensor(
                clip_prod[:], clip_sb[:, 0:1], clip_sb[:, 1:2],
                mybir.AluOpType.mult,
            )

            # SxT[ki, ko, m] = sign(x[m, ko*P + ki]); SyT likewise for y.
            SxT = sxtp.tile([P, KO, MSH], op_dt)
            SyT = sytp.tile([P, KO, NSH], op_dt)

            def prep(src_dram, ko, dst):
                st = stagep.tile([P, MSH], f32, name="st", tag="stage")
                nc.sync.dma_start(st[:], src_dram[ts(ko, P), :])
                nc.scalar.activation(dst, st[:], Sign)

            # Stream k-chunks: x and y interleaved so the first matmuls can
            # start after one chunk of each; the Tile scheduler overlaps the
            # rest of the prep with the matmul wavefront.
            for ko in range(KO):
                prep(xsT, ko, SxT[:, ko, :])
                prep(ysT, ko, SyT[:, ko, :])

            for i in range(MT):
                for nch in range(NCH):
                    ps = psump.tile([P, FD], f32, name="ps")
                    if USE_FP8:
                        for kd in range(KO // 2):
                            nc.tensor.matmul(
                                ps[:],
                                lhsT=SxT[:, 2 * kd : 2 * kd + 2, ts(i, P)],
                                rhs=SyT[:, 2 * kd : 2 * kd + 2, ts(nch, FD)],
                                start=(kd == 0),
                                stop=(kd == KO // 2 - 1),
                                perf_mode=mybir.MatmulPerfMode.DoubleRow,
                            )
                    else:
                        for ko in range(KO):
                            nc.tensor.matmul(
                                ps[:],
                                lhsT=SxT[:, ko, ts(i, P)],
                                rhs=SyT[:, ko, ts(nch, FD)],
                                start=(ko == 0),
                                stop=(ko == KO - 1),
                            )
                    ot = outp.tile([P, FD], f32, name="ot")
                    nc.vector.tensor_scalar_mul(ot[:], ps[:], clip_prod[:])
                    nc.sync.dma_start(out[ts(i, P), ts(nch, FD)], ot[:])

    nc.compile()
    return nc


_PROGRAM_CACHE = None


def _get_program():
    global _PROGRAM_CACHE
    if _PROGRAM_CACHE is None:
        _PROGRAM_CACHE = _build_program()
    return _PROGRAM_CACHE


def _shard_inputs(x, y, x_clip, y_clip):
    x = np.asarray(x, dtype=np.float32)
    y = np.asarray(y, dtype=np.float32)
    clips = np.empty((P, 2), dtype=np.float32)
    clips[:, 0] = np.float32(x_clip)
    clips[:, 1] = np.float32(y_clip)
    in_maps = []
    for c in range(NCORES):
        b, mh, nh = c // 4, (c % 4) // 2, c % 2
        in_maps.append(
            {
                "xsT": np.ascontiguousarray(x[b, mh * MSH : (mh + 1) * MSH, :].T),
                "ysT": np.ascontiguousarray(y[b, nh * NSH : (nh + 1) * NSH, :].T),
                "clips": clips,
            }
        )
    return in_maps


def run_sharded(x, y, x_clip, y_clip, trace=False, **kwargs):
    """Run the SPMD kernel; returns (out, BassKernelResults)."""
    from concourse.bass_utils import run_bass_kernel_spmd

    nc = _get_program()
    in_maps = _shard_inputs(x, y, x_clip, y_clip)
    res = run_bass_kernel_spmd(
        nc, in_maps, core_ids=list(range(NCORES)), trace=trace, **kwargs
    )
    out = np.empty((B, M, N), dtype=np.float32)
    for c in range(NCORES):
        b, mh, nh = c // 4, (c % 4) // 2, c % 2
        out[b, mh * MSH : (mh + 1) * MSH, nh * NSH : (nh + 1) * NSH] = res.results[
            c
        ]["out"]
    return out, res


def kernel(x, y, x_clip, y_clip):
    out, _ = run_sharded(x, y, x_clip, y_clip, trace=False)
    return out
